# revision 3
# baseline (speedup 1.0000x reference)
"""MATCC kernel for 8 Trainium2 NeuronCores.

Sharding (per spec hint): SAttention couples the batch (stock) dim, so the
B x B attention is sharded over the 16 independent (time, head) pairs --
core t computes both heads of time t. The device kernel runs the dominant
34 GFLOP attention (scores matmul + softmax + PV matmul); the small
row-wise layers (~1 GFLOP, memory-bound) run vectorized on host.

Device math per (t, h) pair, B=2048, dh=128:
  S^T[c,b] = K^T(dh,c).T @ Q^T(dh,b)  via fp32r matmuls (N=512 strips)
  E^T = exp(S^T / TEMP)  (ScalarE, bf16 out; scores ~ +-0.6 so no max-sub)
  A[b,:] , rowsum[b] = E^T.T @ [V | 1]  (bf16 matmuls, ones column fused)
  att = A / rowsum  (per-partition reciprocal * scale on eviction)
"""

import os
import sys

sys.path.insert(0, "/opt/trn_rl_repo")

import numpy as np
import ml_dtypes

# ---- model config (hardcoded, mirrors the MATCC reference) ----
B = 2048
SEQ = 8
D_FEAT = 158
D_GATE = 63
D_MODEL = 256
S_NHEAD = 2
T_NHEAD = 4
CTX = 300
KER, STR = 5, 5
TEMP = float(np.sqrt(D_MODEL / S_NHEAD))
N_CORES = 8
DH = D_MODEL // S_NHEAD  # 128

_COMPILED = {}


def _lin(x, w, b=None):
    y = x @ w.T
    return y if b is None else y + b


def _ln(x, g, b, eps=1e-5):
    m = x.mean(-1, keepdims=True)
    v = ((x - m) ** 2).mean(-1, keepdims=True)
    return (x - m) / np.sqrt(v + eps) * g + b


def _tshift_half(x):
    half = x.shape[-1] // 2
    sh = np.concatenate([np.zeros_like(x[:, :1, :half]), x[:, :-1, :half]], axis=1)
    return np.concatenate([sh, x[:, :, half:]], axis=-1)


def _sigmoid(x):
    return 1.0 / (1.0 + np.exp(-x))


def _timemix(x, p):
    Bn, T, C = x.shape
    H, hs = T_NHEAD, C // T_NHEAD
    xs = _tshift_half(x)
    k = np.exp(np.minimum(_lin(xs, p["tm_k_w"], p["tm_k_b"]), 30.0))
    v = _lin(xs, p["tm_v_w"], p["tm_v_b"])
    r = _lin(xs, p["tm_r_w"], p["tm_r_b"])
    sum_k = np.cumsum(k, axis=1)
    tw_pad = np.concatenate([p["time_w"], np.zeros_like(p["time_w"])], axis=1)
    idx = (CTX - 1) - np.arange(T)[:, None] + np.arange(T)[None, :]
    w = tw_pad[:, idx]
    w = w * p["time_alpha"][:, :, :T] * p["time_beta"][:, :T, :]
    kv = (k * v).reshape(Bn, T, H, hs)
    wkv = np.einsum("htu,buhc->bthc", w, kv).reshape(Bn, T, C)
    rwkv = _sigmoid(r) * wkv / sum_k
    return _lin(rwkv, p["tm_o_w"], p["tm_o_b"]) * p["time_gamma"][:T, :]


def _chanmix(x, p):
    xs = _tshift_half(x)
    k = _lin(xs, p["cm_k_w"], p["cm_k_b"])
    v = _lin(xs, p["cm_v_w"], p["cm_v_b"])
    r = _lin(xs, p["cm_r_w"], p["cm_r_b"])
    sp = np.logaddexp(0.0, k)  # softplus
    mish_k = k * np.tanh(sp)
    wkv = _lin(mish_k * v, p["cm_w_w"], p["cm_w_b"])
    return _sigmoid(r) * wkv


def _build_attn_kernel():
    """Bass kernel: per core, 2 (t,h) attention pairs over the full B."""
    import concourse.bass as bass
    import concourse.tile as tile
    import concourse.mybir as mybir
    from concourse import bacc

    nc = bacc.Bacc("TRN2", target_bir_lowering=False, debug=False,
                   num_devices=N_CORES)
    f32, f32r, bf16 = mybir.dt.float32, mybir.dt.float32r, mybir.dt.bfloat16

    qT = nc.declare_dram_parameter("qT", [2, 128, B], f32r, isOutput=False)
    kT = nc.declare_dram_parameter("kT", [2, 128, B], f32r, isOutput=False)
    vA = nc.declare_dram_parameter("vA", [2, 16, 128, 128], bf16, isOutput=False)
    att = nc.declare_dram_parameter("att", [2, B, 128], f32, isOutput=True)

    NC_CH = 16  # 2048 / 128 c-chunks
    NS = 4      # 2048 / 512 b-strips

    with tile.TileContext(nc) as tc:
        with (
            tc.tile_pool(name="inp", bufs=1) as inp,
            tc.tile_pool(name="es", bufs=2) as es,
            tc.tile_pool(name="outp", bufs=4) as outp,
            tc.tile_pool(name="ps_s", bufs=4, space="PSUM") as ps_s,
            tc.tile_pool(name="ps_a", bufs=4, space="PSUM") as ps_a,
        ):
            qs = inp.tile([128, 2, B], f32r)
            ks = inp.tile([128, 2, B], f32r)
            vs = inp.tile([128, 2, NC_CH, 129], bf16)
            nc.vector.memset(vs[:], 1.0)
            for p in range(2):
                nc.sync.dma_start(qs[:, p, :], qT[p])
                nc.sync.dma_start(ks[:, p, :], kT[p])
                for c in range(NC_CH):
                    nc.sync.dma_start(vs[:, p, c, :128], vA[p, c])

            for p in range(2):
                for s in range(NS):
                    e_strip = es.tile([128, NC_CH, 512], bf16)
                    for c in range(NC_CH):
                        ps = ps_s.tile([128, 512], f32)
                        nc.tensor.matmul(
                            ps[:],
                            ks[:, p, c * 128:(c + 1) * 128],
                            qs[:, p, s * 512:(s + 1) * 512],
                            start=True, stop=True,
                        )
                        nc.scalar.activation(
                            e_strip[:, c, :], ps[:],
                            mybir.ActivationFunctionType.Exp,
                            scale=1.0 / TEMP,
                        )
                    for bs in range(4):
                        pa = ps_a.tile([128, 129], f32)
                        for c in range(NC_CH):
                            nc.tensor.matmul(
                                pa[:],
                                e_strip[:, c, bs * 128:(bs + 1) * 128],
                                vs[:, p, c, :],
                                start=(c == 0), stop=(c == NC_CH - 1),
                            )
                        rec = outp.tile([128, 1], f32)
                        nc.vector.reciprocal(rec[:], pa[:, 128:129])
                        at = outp.tile([128, 128], f32)
                        nc.vector.tensor_scalar_mul(at[:], pa[:, :128], rec[:])
                        b0 = (s * 4 + bs) * 128
                        nc.sync.dma_start(att[p, b0:b0 + 128, :], at[:])
    nc.compile()
    return nc


def _run_attention(q, k, v):
    """q,k,v: [T=8, B, H=2, dh=128] fp32. Returns att [T, B, H, dh]."""
    from concourse.bass_utils import run_bass_kernel_spmd

    if "nc" not in _COMPILED:
        _COMPILED["nc"] = _build_attn_kernel()
    nc = _COMPILED["nc"]

    in_maps = []
    for t in range(N_CORES):
        qTt = np.ascontiguousarray(q[t].transpose(1, 2, 0))  # [H, dh, B]
        kTt = np.ascontiguousarray(k[t].transpose(1, 2, 0))
        vt = v[t].transpose(1, 0, 2)  # [H, B, dh]
        vA = np.ascontiguousarray(
            vt.reshape(2, 16, 128, 128)).astype(ml_dtypes.bfloat16)
        in_maps.append({"qT": qTt.astype(np.float32), "kT": kTt.astype(np.float32),
                        "vA": vA})

    trace = bool(os.environ.get("MATCC_TRACE"))
    import time as _time
    try:
        _t0 = _time.time()
        res = run_bass_kernel_spmd(nc, in_maps, core_ids=list(range(N_CORES)),
                                   trace=trace)
    except ModuleNotFoundError:
        _t0 = _time.time()
        res = run_bass_kernel_spmd(nc, in_maps, core_ids=list(range(N_CORES)))
    _COMPILED["last_wall_s"] = _time.time() - _t0
    _COMPILED["last_exec_ns"] = res.exec_time_ns
    att = np.empty((SEQ, B, S_NHEAD, DH), dtype=np.float32)
    for t in range(N_CORES):
        a = res.results[t]["att"]  # [2, B, 128]
        att[t, :, 0, :] = a[0]
        att[t, :, 1, :] = a[1]
    return att


def kernel(x, params):
    p = {kk: np.asarray(vv, dtype=np.float32) for kk, vv in params.items()}
    x = np.asarray(x, dtype=np.float32)

    src = x[:, :, :D_FEAT]
    gate = x[:, :, D_FEAT:D_FEAT + D_GATE]
    # Filter: Linear -> depthwise conv (K=5, S=5, out len 1) -> Linear
    g = _lin(gate, p["filt_trans_w"], p["filt_trans_b"])          # [B,T,158]
    gt = g.transpose(0, 2, 1)                                      # [B,158,T]
    agg = np.einsum("bck,ck->bc", gt[:, :, :KER], p["filt_conv_w"][:, 0, :])
    agg = agg + p["filt_conv_b"]                                   # [B,158]
    gout = agg * p["filt_proj_w"][0, 0] + p["filt_proj_b"][0]      # [B,158]
    src = src + gout[:, None, :]
    # input projection
    h = _lin(src, p["in_w"], p["in_b"])                            # [B,T,D]
    # DLinear
    hp = np.concatenate([h[:, :1], h, h[:, -1:]], axis=1)
    trend = (hp[:, :-2] + hp[:, 1:-1] + hp[:, 2:]) / 3.0
    seas = h - trend
    h = (_lin(seas.transpose(0, 2, 1), p["dl_seas_w"], p["dl_seas_b"])
         + _lin(trend.transpose(0, 2, 1), p["dl_trend_w"], p["dl_trend_b"])
         ).transpose(0, 2, 1)
    # RWKV block
    h = h + _timemix(_ln(h, p["ln1_g"], p["ln1_b"]), p)
    h = h + _chanmix(_ln(h, p["ln2_g"], p["ln2_b"]), p)
    # SAttention (device): attention across the stock dim per (t, h) pair
    xs = _ln(h, p["sa_ln1_g"], p["sa_ln1_b"])
    q = _lin(xs, p["q_w"]).transpose(1, 0, 2)                      # [T,B,D]
    k = _lin(xs, p["k_w"]).transpose(1, 0, 2)
    v = _lin(xs, p["v_w"]).transpose(1, 0, 2)
    qh = q.reshape(SEQ, B, S_NHEAD, DH)
    kh = k.reshape(SEQ, B, S_NHEAD, DH)
    vh = v.reshape(SEQ, B, S_NHEAD, DH)
    ath = _run_attention(qh, kh, vh)                               # [T,B,H,dh]
    att = ath.reshape(SEQ, B, D_MODEL).transpose(1, 0, 2)          # [B,T,D]
    xt = _ln(xs + att, p["sa_ln2_g"], p["sa_ln2_b"])
    ff = _lin(np.maximum(_lin(xt, p["ffn1_w"], p["ffn1_b"]), 0.0),
              p["ffn2_w"], p["ffn2_b"])
    h = xt + ff
    # TemporalAttention
    ht = _lin(h, p["ta_w"])                                        # [B,T,D]
    sc = np.einsum("btd,bd->bt", ht, ht[:, -1, :])
    sc = sc - sc.max(axis=1, keepdims=True)
    e = np.exp(sc)
    lam = e / e.sum(axis=1, keepdims=True)
    feat = np.einsum("bt,btd->bd", lam, h)
    return _lin(feat, p["dec_w"], p["dec_b"]).astype(np.float32)
